# revision 56
# baseline (speedup 1.0000x reference)
"""Trainium2 Bass kernel for nn_MultiHeadDistanceLayer.

Math: out[b,k,h] = pool3(S[h,b,:])[k] where
  S[h,b,k'] = sum_{q>=k'} v[h,b,q] * softmax(QK^T/sqrt(D))[q,k']
(the final sum over the query axis commutes with the W=3 key-axis average
pool, so the device only produces the length-L column-sum vector S per
(head, batch); pooling/normalization is a trivial host epilogue).

Sharding: 16 (head, batch) pairs; core c handles batch c//4 and heads
(2*(c%4), 2*(c%4)+1). The tiny O(L*C*D) Q/K/v projections run on the host
(0.8% of FLOPs); the device does the O(L^2) work: scores, softmax, and
causal weighted column sums.

v2: the baseline was Scalar-engine-bound (ACT busy 85%: 64 exp ACTIVATEs
+ 32 accumulator reads ~= the whole 93us kernel). This version splits
each q-tile's 2048-wide exp row across two engines (93.3us -> 75.4us):
  - ACT: native exp on cols 0-1535 (a 3-bank PSUM tile, one ACTIVATE)
    with the fused accumulator giving that range's softmax-denominator
    partial (one ACTIVATION_READ_ACCUMULATOR per tile).
  - DVE: Schraudolph fast-exp on cols 1536-2047 (1-bank PSUM tile,
    single-buffered): one TENSOR_SCALAR computing
    round(s*(128/ln2)*SCALE + 16248.5) with int16 output conversion; the
    int16 bit patterns ARE bf16 exp values (free bitcast). Mean-centered
    constant; ~2% per-element sawtooth error that averages out in the
    512-term denominator partials and in the column sums (those columns
    are mostly the post-softmax-masked region anyway: measured output
    rel_l2 3.2e-3 vs 2.3e-3 for the all-exact baseline). A DVE
    tensor_reduce of that range gives the remaining denominator partial.
Z-combine + reciprocal run batched over groups of 4 tiles on DVE; the
4 diagonal-block causal mask multiplies of a group run as one strided
TT (slot stride 2048 + in-tile step 128 = uniform stride 2176), and the
4 colsum weight tiles of a group are built in one TT: host-precomputed
vpat = pat32*v times a stride-0 broadcast view of the reciprocals
(replaces 4 overhead-dominated TENSOR_SCALARs, -7us DVE).
Colsum matmuls (32-strip wpat trick, PSUM accumulation with start=False)
are unchanged from the baseline, but deferred colsums are issued AFTER
the next tile's scA matmuls + ACTIVATE in the PE queue so the Scalar
engine's next input is never queued behind them (-1.1us).
Pool engine (GPSIMD) rejects all scalar/accum/reduce ops in walrus, so
it only does memsets/input DMA; putting work there (or batching the
reduces, or pre-chaining the tail z-ops) measured slower — the DVE
queue is in-order and any op that waits on ACT's accumulator mid-stream
convoys the whole pipeline.
"""

import sys

for _p in ("/opt/trn_rl_repo",):
    if _p not in sys.path:
        sys.path.insert(0, _p)

import numpy as np

B, L, C = 2, 2048, 256
H, D, W = 8, 32, 3
NCORES = 8
NT = L // 128          # 16 q-tiles per head
SCALE = float(D) ** -0.5

# Schraudolph bf16 fast-exp: pattern16 = round(x*A16 + B16), bitcast bf16
A16 = 128.0 / float(np.log(2.0))
B16 = 127.0 * 128.0 - 7.5          # mean-centered bias (round-to-nearest)

TRACE = False
LAST_EXEC_NS = None
LAST_RES = None
_COMPILED = None
DEBUG_Z = False


def _build():
    import concourse.bacc as bacc
    import concourse.tile as tile
    from concourse import mybir

    f32 = mybir.dt.float32
    bf16 = mybir.dt.bfloat16
    i16 = mybir.dt.int16
    AF = mybir.ActivationFunctionType
    ALU = mybir.AluOpType
    AX = mybir.AxisListType

    nc = bacc.Bacc("TRN2", target_bir_lowering=False, debug=False,
                   num_devices=NCORES)

    # host-projected Q/K, transposed + bf16: rows [QT_h0, KT_h0, QT_h1, KT_h1]
    qk4 = nc.dram_tensor("qk4", [4, 32, L], bf16, kind="ExternalInput")
    # host-precomputed pat32 * v: vpat[p, 32*(NT*hh+t) + c] strips
    vpat = nc.dram_tensor("vpat", [128, 2 * NT * 32], bf16,
                          kind="ExternalInput")
    # 128-wide causal mask for the true-diagonal block (tiled x4): keep j <= p
    msk = nc.dram_tensor("msk", [128, 512], bf16, kind="ExternalInput")
    sout = nc.dram_tensor("sout", [2, 32, L], f32, kind="ExternalOutput")
    if DEBUG_Z:
        zdbg = nc.dram_tensor("zdbg", [128, 32], f32, kind="ExternalOutput")

    with tile.TileContext(nc) as tc:
        with (
            tc.tile_pool(name="big", bufs=1) as big,
            tc.tile_pool(name="qkp", bufs=2) as qkp,
            tc.tile_pool(name="zpool", bufs=2) as zpool,
            tc.tile_pool(name="small", bufs=8) as small,
            tc.tile_pool(name="empool", bufs=8) as empool,
            tc.tile_pool(name="ssbp", bufs=2) as ssbp,
            tc.tile_pool(name="psca", bufs=2, space="PSUM") as psca,
            tc.tile_pool(name="pscb", bufs=1, space="PSUM") as pscb,
            tc.tile_pool(name="psacc", bufs=1, space="PSUM") as psacc,
        ):
            # --- per-head K-padded Q/K scratch, zeroed first (rows 32+
            # must be zero; K=32 matmuls do not register as PE activity
            # for the HAM clock gate, K=128 do) ---
            qkts = []
            for hh in range(2):
                qts = qkp.tile([128, L], bf16, tag=f"qts{hh}", name=f"qts{hh}")
                kts = qkp.tile([128, L], bf16, tag=f"kts{hh}", name=f"kts{hh}")
                qkts.append((qts, kts))

            # one big 8-slot rotating exp buffer (manual rotation so slots
            # have known uniform layout; 512 extra cols pad the strided
            # batched-em rearrange view of the odd slot groups)
            etall = big.tile([128, 18432], bf16, tag="etall", name="etall")

            # --- exp table preload (hidden under input DMA) ---
            warm = big.tile([128, 1], f32, tag="warm")
            nc.vector.memset(warm, 0.0)
            nc.scalar.activation(out=warm, in_=warm, func=AF.Exp)
            # --- PE warmup: ~4us of dense K=128 matmuls during the DMA
            # wait trips the HAM activity window so the first real tiles
            # run at 2.4GHz instead of 1.2GHz
            wrmt = big.tile([128, 512], bf16, tag="wrmt")
            nc.gpsimd.memset(wrmt.bitcast(mybir.dt.uint32), 0)
            wrmp = psacc.tile([128, 512], f32, tag="sacc", name="wrmp")
            for i in range(4):
                nc.tensor.matmul(wrmp, wrmt[:, 0:128], wrmt,
                                 start=True, stop=True)

            # --- zero the K-pad rows, then DMA Q/K into rows 0-31 ---
            k0, q0 = qkts[0][1], qkts[0][0]
            nc.vector.memset(k0.bitcast(mybir.dt.uint32)[:, 0:256], 0)
            nc.gpsimd.memset(q0.bitcast(mybir.dt.uint32)[:, 0:64], 0)
            nc.sync.dma_start(out=k0[0:32, 0:512], in_=qk4[1][:, 0:512])
            nc.scalar.dma_start(out=q0[0:32, 0:128], in_=qk4[0][:, 0:128])
            nc.vector.memset(k0.bitcast(mybir.dt.uint32)[:, 256:1024], 0)
            nc.gpsimd.memset(q0.bitcast(mybir.dt.uint32)[:, 64:1024], 0)
            nc.sync.dma_start(out=k0[0:32, 512:2048], in_=qk4[1][:, 512:2048])
            nc.scalar.dma_start(out=q0[0:32, 128:2048],
                                in_=qk4[0][:, 128:2048])
            nc.vector.memset(qkts[1][1].bitcast(mybir.dt.uint32), 0)
            nc.gpsimd.memset(qkts[1][0].bitcast(mybir.dt.uint32), 0)
            nc.sync.dma_start(out=qkts[1][1][0:32, :], in_=qk4[3])
            nc.scalar.dma_start(out=qkts[1][0][0:32, :], in_=qk4[2])
            vpat_sb = big.tile([128, 2 * NT * 32], bf16, tag="vpat")
            nc.gpsimd.dma_start(out=vpat_sb, in_=vpat[:, :])
            # mask pre-tiled x4 by the host for the batched diag multiply
            msk_sb = big.tile([128, 512], bf16, tag="msk")
            nc.gpsimd.dma_start(out=msk_sb, in_=msk[:, :])

            for hh in range(2):
                qts, kts = qkts[hh]
                sacc = psacc.tile([128, 512], f32, tag="sacc", name="sacc")
                saccs = [sacc[32 * c:32 * (c + 1), :] for c in range(4)]
                # explicit zero-init so every colsum matmul can accumulate
                # (start=False): a start=True on a sub-range of a live bank
                # corrupts the other columns (measured)
                nc.vector.memset(sacc, 0.0)
                zab = zpool.tile([128, NT], f32, tag="zab")
                zgb = zpool.tile([128, NT], f32, tag="zgb")
                zv = zpool.tile([128, NT], f32, tag="zv")
                zr = zpool.tile([128, NT], f32, tag="zr")
                ets = [None] * NT
                ems = [None] * NT
                wps = [None] * NT

                def issue_zchain(g):
                    c0, c1 = 4 * g, 4 * g + 4
                    nc.vector.scalar_tensor_tensor(
                        out=zv[:, c0:c1], in0=zab[:, c0:c1], scalar=1.0,
                        in1=zgb[:, c0:c1], op0=ALU.mult, op1=ALU.add)
                    nc.vector.reciprocal(zr[:, c0:c1], zv[:, c0:c1])
                    # all 4 weight tiles of the group in one TT against a
                    # stride-0 broadcast view of the reciprocals
                    wp4 = small.tile([128, 128], bf16, tag="wpat4")
                    vb = zr[:, c0:c1].unsqueeze(-1).broadcast_to([128, 4, 32])
                    iv0 = 32 * (NT * hh + c0)
                    nc.vector.tensor_tensor(
                        out=wp4.rearrange("p (g c) -> p g c", c=32),
                        in0=vpat_sb[:, iv0:iv0 + 128].rearrange(
                            "p (g c) -> p g c", c=32),
                        in1=vb, op=ALU.mult)
                    for i in range(4):
                        wps[c0 + i] = wp4[:, 32 * i:32 * (i + 1)]

                def issue_em4(g, eng=None):
                    # batched mask-multiply of the 4 diagonal 128-blocks of
                    # group g: slot stride 2048 + in-tile step 128 = uniform
                    # stride 2176 -> one strided TT (on gpsimd, which is
                    # otherwise idle)
                    base = 2048 * ((4 * g) % 8) + 512 * g
                    src = etall[:, base:base + 8704].rearrange(
                        "p (g c) -> p g c", c=2176)[:, :, 0:128]
                    em4 = empool.tile([128, 512], bf16, tag="em4")
                    (eng or nc.vector).tensor_tensor(
                        out=em4.rearrange("p (g c) -> p g c", c=128),
                        in0=src,
                        in1=msk_sb.rearrange("p (g c) -> p g c", c=128),
                        op=ALU.mult)
                    for i in range(4):
                        ems[4 * g + i] = em4[:, 128 * i:128 * (i + 1)]

                def issue_colsum(t, last):
                    cb, s = t // 4, t % 4
                    wp, et, em = wps[t], ets[t], ems[t]
                    for c2 in range(cb):
                        nc.tensor.matmul(saccs[c2], wp,
                                         et[:, 512 * c2:512 * (c2 + 1)],
                                         start=False, stop=last,
                                         tile_position=(0, 32 * c2),
                                         skip_group_check=True)
                    if s > 0:
                        nc.tensor.matmul(saccs[cb][:, 0:128 * s], wp,
                                         et[:, 512 * cb:512 * cb + 128 * s],
                                         start=False, stop=last,
                                         tile_position=(0, 32 * cb),
                                         skip_group_check=True)
                    nc.tensor.matmul(saccs[cb][:, 128 * s:128 * (s + 1)],
                                     wp, em,
                                     start=False, stop=last,
                                     tile_position=(0, 32 * cb),
                                     skip_group_check=True)

                pending_cs = []
                for t in range(NT):
                    lhs = qts[:, 128 * t:128 * (t + 1)]
                    o = 2048 * (t % 8)
                    et = etall[:, o:o + 2048]
                    ets[t] = et
                    scA = psca.tile([128, 1536], f32, tag="sca")
                    scB = pscb.tile([128, 512], f32, tag="scb")
                    nc.tensor.matmul(scA[:, 0:512], lhs, kts[:, 0:512],
                                     start=True, stop=True)
                    nc.tensor.matmul(scA[:, 512:1024], lhs, kts[:, 512:1024],
                                     start=True, stop=True)
                    nc.tensor.matmul(scA[:, 1024:1536], lhs,
                                     kts[:, 1024:1536],
                                     start=True, stop=True)
                    # ACT: exact exp on cols 0-1535 + fused Z accumulator
                    nc.scalar.activation(out=et[:, 0:1536], in_=scA,
                                         func=AF.Exp, scale=SCALE,
                                         accum_out=zab[:, t:t + 1])
                    # colsums deferred from the previous tile go here: after
                    # this tile's scA matmuls (so ACT's next input is never
                    # queued behind them on the PE) but before scB
                    for tc_, last_ in pending_cs:
                        issue_colsum(tc_, last_)
                    pending_cs = []
                    nc.tensor.matmul(scB, lhs, kts[:, 1536:2048],
                                     start=True, stop=True)
                    # DVE: Schraudolph fast-exp on cols 1536-2047 (int16
                    # convert of the affine; bit patterns are bf16 values)
                    nc.vector.tensor_scalar(
                        out=et[:, 1536:2048].bitcast(i16), in0=scB,
                        scalar1=float(A16 * SCALE), scalar2=float(B16),
                        op0=ALU.mult, op1=ALU.add)
                    # DVE: that region's denominator partial
                    nc.vector.tensor_reduce(out=zgb[:, t:t + 1],
                                            in_=et[:, 1536:2048],
                                            axis=AX.X, op=ALU.add)
                    # deferred work, scheduled to keep engines fed:
                    r = t % 4
                    if r == 0 and t >= 4:
                        issue_em4((t - 4) // 4)
                    elif r == 1 and t >= 5:
                        issue_zchain((t - 5) // 4)
                    elif r == 2 and t >= 6:
                        g = (t - 6) // 4
                        pending_cs = [(4 * g, False), (4 * g + 1, False)]
                    elif r == 3 and t >= 7:
                        g = (t - 7) // 4
                        pending_cs = [(4 * g + 2, False), (4 * g + 3, False)]
                for tc_, last_ in pending_cs:
                    issue_colsum(tc_, last_)
                # tail: finish last tiles (groups 0-2 fully issued in-loop);
                # z-chain first so the PE starts the tail colsums' chunk
                # matmuls while the em4 mask-multiply still runs
                issue_zchain(3)
                issue_em4(3, eng=nc.vector)
                issue_colsum(12, False)
                issue_colsum(13, False)
                issue_colsum(14, False)
                issue_colsum(15, True)
                if DEBUG_Z and hh == 0:
                    nc.sync.dma_start(out=zdbg[:, 0:NT], in_=zab[:, 0:NT])
                    nc.sync.dma_start(out=zdbg[:, NT:2 * NT],
                                      in_=zgb[:, 0:NT])
                ssb = ssbp.tile([32, L], f32, tag="ssb")
                for c in range(4):
                    if hh == 1 and c % 2 == 0:
                        # final head only: ACT is idle after its last exp
                        nc.scalar.activation(
                            out=ssb[:, 512 * c:512 * (c + 1)], in_=saccs[c],
                            func=AF.Copy)
                    else:
                        nc.vector.tensor_copy(
                            out=ssb[:, 512 * c:512 * (c + 1)], in_=saccs[c])
                    # ship each chunk as soon as it drains instead of one
                    # DMA after all four copies
                    nc.sync.dma_start(out=sout[hh][:, 512 * c:512 * (c + 1)],
                                      in_=ssb[:, 512 * c:512 * (c + 1)])

    nc.compile()
    return nc


def _get_compiled():
    global _COMPILED
    if _COMPILED is None:
        _COMPILED = _build()
    return _COMPILED


def make_in_maps(x, Wq, bq, Wk, bk, Wv, pe):
    """Host-side sharding: build the per-core input dicts."""
    import ml_dtypes

    x = np.asarray(x, np.float32)
    Wq = np.asarray(Wq, np.float32)
    bq = np.asarray(bq, np.float32).reshape(H, D)
    Wk = np.asarray(Wk, np.float32)
    bk = np.asarray(bk, np.float32).reshape(H, D)
    Wv = np.asarray(Wv, np.float32)
    pe = np.asarray(pe, np.float32)

    xq = x + pe[None, :, :]                       # (B, L, C)
    v = np.einsum("blc,ch->blh", x, Wv)           # (B, L, H)
    q_all = (xq @ Wq).reshape(B, L, H, D) + bq[None, None]   # (B, L, H, D)
    k_all = (xq @ Wk).reshape(B, L, H, D) + bk[None, None]

    p_idx = np.arange(128)
    pat32 = (p_idx[:, None] // 4 == np.arange(32)[None, :]).astype(np.float32)
    msk = np.tile((np.arange(128)[None, :] <= p_idx[:, None]).astype(
        ml_dtypes.bfloat16), (1, 4))

    in_maps = []
    for core in range(NCORES):
        b = core // 4
        h0 = 2 * (core % 4)
        qk4 = np.empty((4, 32, L), np.float32)
        for hh in range(2):
            qk4[2 * hh] = q_all[b, :, h0 + hh, :].T
            qk4[2 * hh + 1] = k_all[b, :, h0 + hh, :].T
        qk4 = qk4.astype(ml_dtypes.bfloat16)
        vnat = np.empty((128, 2 * NT), np.float32)
        for hh in range(2):
            # vnat[p, NT*hh + t] = v[b, 128*t + p, h0+hh]
            vnat[:, NT * hh:NT * (hh + 1)] = v[b, :, h0 + hh].reshape(NT, 128).T
        # vpat[p, 32*iv + c] = pat32[p, c] * vnat[p, iv]
        vpat = (vnat[:, :, None] * pat32[:, None, :]).reshape(
            128, 2 * NT * 32).astype(ml_dtypes.bfloat16)
        in_maps.append(dict(qk4=qk4, vpat=vpat, msk=msk))
    return in_maps


def postprocess(results):
    """Host-side gather: strip-sum, W=3 same-pool, assemble (B, L, H)."""
    S = np.zeros((H, B, L), np.float32)
    for core in range(NCORES):
        b = core // 4
        h0 = 2 * (core % 4)
        sraw = np.asarray(results[core]["sout"], np.float32)  # (2, 32, L)
        for hh in range(2):
            S[h0 + hh, b, :] = sraw[hh].sum(axis=0)
    Sp = np.pad(S, ((0, 0), (0, 0), (1, 1)))
    sums = Sp[:, :, :-2] + Sp[:, :, 1:-1] + Sp[:, :, 2:]
    counts = np.full(L, float(W), np.float32)
    counts[0] = counts[-1] = W - 1
    pooled = sums / counts[None, None, :]
    return np.ascontiguousarray(pooled.transpose(1, 2, 0)).astype(np.float32)


def kernel(x, Wq, bq, Wk, bk, Wv, pe):
    global LAST_EXEC_NS, LAST_RES
    from concourse.bass_utils import run_bass_kernel_spmd

    nc = _get_compiled()
    in_maps = make_in_maps(x, Wq, bq, Wk, bk, Wv, pe)
    res = run_bass_kernel_spmd(nc, in_maps, list(range(NCORES)), trace=TRACE)
    LAST_EXEC_NS = res.exec_time_ns
    LAST_RES = res
    return postprocess(res.results)


# revision 57
# speedup vs baseline: 1.0167x; 1.0167x over previous
"""Trainium2 Bass kernel for nn_MultiHeadDistanceLayer.

Math: out[b,k,h] = pool3(S[h,b,:])[k] where
  S[h,b,k'] = sum_{q>=k'} v[h,b,q] * softmax(QK^T/sqrt(D))[q,k']
(the final sum over the query axis commutes with the W=3 key-axis average
pool, so the device only produces the length-L column-sum vector S per
(head, batch); pooling/normalization is a trivial host epilogue).

Sharding: 16 (head, batch) pairs; core c handles batch c//4 and heads
(2*(c%4), 2*(c%4)+1). The tiny O(L*C*D) Q/K/v projections run on the host
(0.8% of FLOPs); the device does the O(L^2) work: scores, softmax, and
causal weighted column sums.

v2: the baseline was Scalar-engine-bound (ACT busy 85%: 64 exp ACTIVATEs
+ 32 accumulator reads ~= the whole 93us kernel). This version splits
each q-tile's 2048-wide exp row across two engines (93.3us -> 75.4us):
  - ACT: native exp on cols 0-1535 (a 3-bank PSUM tile, one ACTIVATE)
    with the fused accumulator giving that range's softmax-denominator
    partial (one ACTIVATION_READ_ACCUMULATOR per tile).
  - DVE: Schraudolph fast-exp on cols 1536-2047 (1-bank PSUM tile,
    single-buffered): one TENSOR_SCALAR computing
    round(s*(128/ln2)*SCALE + 16248.5) with int16 output conversion; the
    int16 bit patterns ARE bf16 exp values (free bitcast). Mean-centered
    constant; ~2% per-element sawtooth error that averages out in the
    512-term denominator partials and in the column sums (those columns
    are mostly the post-softmax-masked region anyway: measured output
    rel_l2 3.2e-3 vs 2.3e-3 for the all-exact baseline). A DVE
    tensor_reduce of that range gives the remaining denominator partial.
Z-combine + reciprocal run batched over groups of 4 tiles on DVE; the
4 diagonal-block causal mask multiplies of a group run as one strided
TT (slot stride 2048 + in-tile step 128 = uniform stride 2176), and the
4 colsum weight tiles of a group are built in one TT: host-precomputed
vpat = pat32*v times a stride-0 broadcast view of the reciprocals
(replaces 4 overhead-dominated TENSOR_SCALARs, -7us DVE).
Colsum matmuls (32-strip wpat trick, PSUM accumulation with start=False)
are unchanged from the baseline, but deferred colsums are issued AFTER
the next tile's scA matmuls + ACTIVATE in the PE queue so the Scalar
engine's next input is never queued behind them (-1.1us).
Pool engine (GPSIMD) rejects all scalar/accum/reduce ops in walrus, so
it only does memsets/input DMA; putting work there (or batching the
reduces, or pre-chaining the tail z-ops) measured slower — the DVE
queue is in-order and any op that waits on ACT's accumulator mid-stream
convoys the whole pipeline.
"""

import sys

for _p in ("/opt/trn_rl_repo",):
    if _p not in sys.path:
        sys.path.insert(0, _p)

import numpy as np

B, L, C = 2, 2048, 256
H, D, W = 8, 32, 3
NCORES = 8
NT = L // 128          # 16 q-tiles per head
SCALE = float(D) ** -0.5

# Schraudolph bf16 fast-exp: pattern16 = round(x*A16 + B16), bitcast bf16
A16 = 128.0 / float(np.log(2.0))
B16 = 127.0 * 128.0 - 7.5          # mean-centered bias (round-to-nearest)

TRACE = False
LAST_EXEC_NS = None
LAST_RES = None
_COMPILED = None
DEBUG_Z = False


def _build():
    import concourse.bacc as bacc
    import concourse.tile as tile
    from concourse import mybir

    f32 = mybir.dt.float32
    bf16 = mybir.dt.bfloat16
    i16 = mybir.dt.int16
    AF = mybir.ActivationFunctionType
    ALU = mybir.AluOpType
    AX = mybir.AxisListType

    nc = bacc.Bacc("TRN2", target_bir_lowering=False, debug=False,
                   num_devices=NCORES)

    # host-projected Q/K, transposed + bf16: rows [QT_h0, KT_h0, QT_h1, KT_h1]
    qk4 = nc.dram_tensor("qk4", [4, 32, L], bf16, kind="ExternalInput")
    # host-precomputed pat32 * v: vpat[p, 32*(NT*hh+t) + c] strips
    vpat = nc.dram_tensor("vpat", [128, 2 * NT * 32], bf16,
                          kind="ExternalInput")
    # 128-wide causal mask for the true-diagonal block (tiled x4): keep j <= p
    msk = nc.dram_tensor("msk", [128, 512], bf16, kind="ExternalInput")
    sout = nc.dram_tensor("sout", [2, 32, L], f32, kind="ExternalOutput")
    if DEBUG_Z:
        zdbg = nc.dram_tensor("zdbg", [128, 32], f32, kind="ExternalOutput")

    with tile.TileContext(nc) as tc:
        with (
            tc.tile_pool(name="big", bufs=1) as big,
            tc.tile_pool(name="qkp", bufs=2) as qkp,
            tc.tile_pool(name="zpool", bufs=2) as zpool,
            tc.tile_pool(name="small", bufs=8) as small,
            tc.tile_pool(name="empool", bufs=8) as empool,
            tc.tile_pool(name="ssbp", bufs=2) as ssbp,
            tc.tile_pool(name="psca", bufs=2, space="PSUM") as psca,
            tc.tile_pool(name="pscb", bufs=1, space="PSUM") as pscb,
            tc.tile_pool(name="psacc", bufs=1, space="PSUM") as psacc,
        ):
            # --- per-head K-padded Q/K scratch, zeroed first (rows 32+
            # must be zero; K=32 matmuls do not register as PE activity
            # for the HAM clock gate, K=128 do) ---
            qkts = []
            for hh in range(2):
                qts = qkp.tile([128, L], bf16, tag=f"qts{hh}", name=f"qts{hh}")
                kts = qkp.tile([128, L], bf16, tag=f"kts{hh}", name=f"kts{hh}")
                qkts.append((qts, kts))

            # one big 8-slot rotating exp buffer (manual rotation so slots
            # have known uniform layout; 512 extra cols pad the strided
            # batched-em rearrange view of the odd slot groups)
            etall = big.tile([128, 18432], bf16, tag="etall", name="etall")

            # --- exp table preload (hidden under input DMA) ---
            warm = big.tile([128, 1], f32, tag="warm")
            nc.vector.memset(warm, 0.0)
            nc.scalar.activation(out=warm, in_=warm, func=AF.Exp)
            # --- PE warmup: ~4us of dense K=128 matmuls during the DMA
            # wait trips the HAM activity window so the first real tiles
            # run at 2.4GHz instead of 1.2GHz
            wrmt = big.tile([128, 512], bf16, tag="wrmt")
            nc.gpsimd.memset(wrmt.bitcast(mybir.dt.uint32), 0)
            wrmp = psacc.tile([128, 512], f32, tag="sacc", name="wrmp")
            # 8 x 512 cols ~= 3.4us at the mid pstate: enough continuous PE
            # busy to trip the HAM ramp to 2.4GHz before the first scores
            # (hidden under the input-DMA wait either way)
            for i in range(8):
                nc.tensor.matmul(wrmp, wrmt[:, 0:128], wrmt,
                                 start=True, stop=True)

            # --- zero the K-pad rows, then DMA Q/K into rows 0-31 ---
            k0, q0 = qkts[0][1], qkts[0][0]
            nc.vector.memset(k0.bitcast(mybir.dt.uint32)[:, 0:256], 0)
            nc.gpsimd.memset(q0.bitcast(mybir.dt.uint32)[:, 0:64], 0)
            nc.sync.dma_start(out=k0[0:32, 0:512], in_=qk4[1][:, 0:512])
            nc.scalar.dma_start(out=q0[0:32, 0:128], in_=qk4[0][:, 0:128])
            nc.vector.memset(k0.bitcast(mybir.dt.uint32)[:, 256:1024], 0)
            nc.gpsimd.memset(q0.bitcast(mybir.dt.uint32)[:, 64:1024], 0)
            nc.sync.dma_start(out=k0[0:32, 512:2048], in_=qk4[1][:, 512:2048])
            nc.scalar.dma_start(out=q0[0:32, 128:2048],
                                in_=qk4[0][:, 128:2048])
            nc.vector.memset(qkts[1][1].bitcast(mybir.dt.uint32), 0)
            nc.gpsimd.memset(qkts[1][0].bitcast(mybir.dt.uint32), 0)
            nc.sync.dma_start(out=qkts[1][1][0:32, :], in_=qk4[3])
            nc.scalar.dma_start(out=qkts[1][0][0:32, :], in_=qk4[2])
            vpat_sb = big.tile([128, 2 * NT * 32], bf16, tag="vpat")
            nc.gpsimd.dma_start(out=vpat_sb, in_=vpat[:, :])
            # mask pre-tiled x4 by the host for the batched diag multiply
            msk_sb = big.tile([128, 512], bf16, tag="msk")
            nc.gpsimd.dma_start(out=msk_sb, in_=msk[:, :])

            for hh in range(2):
                qts, kts = qkts[hh]
                sacc = psacc.tile([128, 512], f32, tag="sacc", name="sacc")
                saccs = [sacc[32 * c:32 * (c + 1), :] for c in range(4)]
                # explicit zero-init so every colsum matmul can accumulate
                # (start=False): a start=True on a sub-range of a live bank
                # corrupts the other columns (measured)
                nc.vector.memset(sacc, 0.0)
                zab = zpool.tile([128, NT], f32, tag="zab")
                zgb = zpool.tile([128, NT], f32, tag="zgb")
                zv = zpool.tile([128, NT], f32, tag="zv")
                zr = zpool.tile([128, NT], f32, tag="zr")
                ets = [None] * NT
                ems = [None] * NT
                wps = [None] * NT

                def issue_zchain(g):
                    c0, c1 = 4 * g, 4 * g + 4
                    nc.vector.scalar_tensor_tensor(
                        out=zv[:, c0:c1], in0=zab[:, c0:c1], scalar=1.0,
                        in1=zgb[:, c0:c1], op0=ALU.mult, op1=ALU.add)
                    nc.vector.reciprocal(zr[:, c0:c1], zv[:, c0:c1])
                    # all 4 weight tiles of the group in one TT against a
                    # stride-0 broadcast view of the reciprocals
                    wp4 = small.tile([128, 128], bf16, tag="wpat4")
                    vb = zr[:, c0:c1].unsqueeze(-1).broadcast_to([128, 4, 32])
                    iv0 = 32 * (NT * hh + c0)
                    nc.vector.tensor_tensor(
                        out=wp4.rearrange("p (g c) -> p g c", c=32),
                        in0=vpat_sb[:, iv0:iv0 + 128].rearrange(
                            "p (g c) -> p g c", c=32),
                        in1=vb, op=ALU.mult)
                    for i in range(4):
                        wps[c0 + i] = wp4[:, 32 * i:32 * (i + 1)]

                def issue_em4(g, eng=None):
                    # batched mask-multiply of the 4 diagonal 128-blocks of
                    # group g: slot stride 2048 + in-tile step 128 = uniform
                    # stride 2176 -> one strided TT (on gpsimd, which is
                    # otherwise idle)
                    base = 2048 * ((4 * g) % 8) + 512 * g
                    src = etall[:, base:base + 8704].rearrange(
                        "p (g c) -> p g c", c=2176)[:, :, 0:128]
                    em4 = empool.tile([128, 512], bf16, tag="em4")
                    (eng or nc.vector).tensor_tensor(
                        out=em4.rearrange("p (g c) -> p g c", c=128),
                        in0=src,
                        in1=msk_sb.rearrange("p (g c) -> p g c", c=128),
                        op=ALU.mult)
                    for i in range(4):
                        ems[4 * g + i] = em4[:, 128 * i:128 * (i + 1)]

                def issue_colsum(t, last):
                    cb, s = t // 4, t % 4
                    wp, et, em = wps[t], ets[t], ems[t]
                    for c2 in range(cb):
                        nc.tensor.matmul(saccs[c2], wp,
                                         et[:, 512 * c2:512 * (c2 + 1)],
                                         start=False, stop=last,
                                         tile_position=(0, 32 * c2),
                                         skip_group_check=True)
                    if s > 0:
                        nc.tensor.matmul(saccs[cb][:, 0:128 * s], wp,
                                         et[:, 512 * cb:512 * cb + 128 * s],
                                         start=False, stop=last,
                                         tile_position=(0, 32 * cb),
                                         skip_group_check=True)
                    nc.tensor.matmul(saccs[cb][:, 128 * s:128 * (s + 1)],
                                     wp, em,
                                     start=False, stop=last,
                                     tile_position=(0, 32 * cb),
                                     skip_group_check=True)

                pending_cs = []
                for t in range(NT):
                    lhs = qts[:, 128 * t:128 * (t + 1)]
                    o = 2048 * (t % 8)
                    et = etall[:, o:o + 2048]
                    ets[t] = et
                    scA = psca.tile([128, 1536], f32, tag="sca")
                    scB = pscb.tile([128, 512], f32, tag="scb")
                    nc.tensor.matmul(scA[:, 0:512], lhs, kts[:, 0:512],
                                     start=True, stop=True)
                    nc.tensor.matmul(scA[:, 512:1024], lhs, kts[:, 512:1024],
                                     start=True, stop=True)
                    nc.tensor.matmul(scA[:, 1024:1536], lhs,
                                     kts[:, 1024:1536],
                                     start=True, stop=True)
                    # ACT: exact exp on cols 0-1535 + fused Z accumulator
                    nc.scalar.activation(out=et[:, 0:1536], in_=scA,
                                         func=AF.Exp, scale=SCALE,
                                         accum_out=zab[:, t:t + 1])
                    # colsums deferred from the previous tile go here: after
                    # this tile's scA matmuls (so ACT's next input is never
                    # queued behind them on the PE) but before scB
                    for tc_, last_ in pending_cs:
                        issue_colsum(tc_, last_)
                    pending_cs = []
                    nc.tensor.matmul(scB, lhs, kts[:, 1536:2048],
                                     start=True, stop=True)
                    # DVE: Schraudolph fast-exp on cols 1536-2047 (int16
                    # convert of the affine; bit patterns are bf16 values)
                    nc.vector.tensor_scalar(
                        out=et[:, 1536:2048].bitcast(i16), in0=scB,
                        scalar1=float(A16 * SCALE), scalar2=float(B16),
                        op0=ALU.mult, op1=ALU.add)
                    # DVE: that region's denominator partial
                    nc.vector.tensor_reduce(out=zgb[:, t:t + 1],
                                            in_=et[:, 1536:2048],
                                            axis=AX.X, op=ALU.add)
                    # deferred work, scheduled to keep engines fed:
                    r = t % 4
                    if r == 0 and t >= 4:
                        issue_em4((t - 4) // 4)
                    elif r == 1 and t >= 5:
                        issue_zchain((t - 5) // 4)
                    elif r == 2 and t >= 6:
                        g = (t - 6) // 4
                        pending_cs = [(4 * g, False), (4 * g + 1, False)]
                    elif r == 3 and t >= 7:
                        g = (t - 7) // 4
                        pending_cs = [(4 * g + 2, False), (4 * g + 3, False)]
                for tc_, last_ in pending_cs:
                    issue_colsum(tc_, last_)
                # tail: finish last tiles (groups 0-2 fully issued in-loop);
                # z-chain first so the PE starts the tail colsums' chunk
                # matmuls while the em4 mask-multiply still runs
                issue_zchain(3)
                issue_em4(3, eng=nc.vector)
                issue_colsum(12, False)
                issue_colsum(13, False)
                issue_colsum(14, False)
                issue_colsum(15, True)
                if DEBUG_Z and hh == 0:
                    nc.sync.dma_start(out=zdbg[:, 0:NT], in_=zab[:, 0:NT])
                    nc.sync.dma_start(out=zdbg[:, NT:2 * NT],
                                      in_=zgb[:, 0:NT])
                ssb = ssbp.tile([32, L], f32, tag="ssb")
                for c in range(4):
                    if hh == 1 and c % 2 == 0:
                        # final head only: ACT is idle after its last exp
                        nc.scalar.activation(
                            out=ssb[:, 512 * c:512 * (c + 1)], in_=saccs[c],
                            func=AF.Copy)
                    else:
                        nc.vector.tensor_copy(
                            out=ssb[:, 512 * c:512 * (c + 1)], in_=saccs[c])
                    # ship each chunk as soon as it drains instead of one
                    # DMA after all four copies
                    nc.sync.dma_start(out=sout[hh][:, 512 * c:512 * (c + 1)],
                                      in_=ssb[:, 512 * c:512 * (c + 1)])

    nc.compile()
    return nc


def _get_compiled():
    global _COMPILED
    if _COMPILED is None:
        _COMPILED = _build()
    return _COMPILED


def make_in_maps(x, Wq, bq, Wk, bk, Wv, pe):
    """Host-side sharding: build the per-core input dicts."""
    import ml_dtypes

    x = np.asarray(x, np.float32)
    Wq = np.asarray(Wq, np.float32)
    bq = np.asarray(bq, np.float32).reshape(H, D)
    Wk = np.asarray(Wk, np.float32)
    bk = np.asarray(bk, np.float32).reshape(H, D)
    Wv = np.asarray(Wv, np.float32)
    pe = np.asarray(pe, np.float32)

    xq = x + pe[None, :, :]                       # (B, L, C)
    v = np.einsum("blc,ch->blh", x, Wv)           # (B, L, H)
    q_all = (xq @ Wq).reshape(B, L, H, D) + bq[None, None]   # (B, L, H, D)
    k_all = (xq @ Wk).reshape(B, L, H, D) + bk[None, None]

    p_idx = np.arange(128)
    pat32 = (p_idx[:, None] // 4 == np.arange(32)[None, :]).astype(np.float32)
    msk = np.tile((np.arange(128)[None, :] <= p_idx[:, None]).astype(
        ml_dtypes.bfloat16), (1, 4))

    in_maps = []
    for core in range(NCORES):
        b = core // 4
        h0 = 2 * (core % 4)
        qk4 = np.empty((4, 32, L), np.float32)
        for hh in range(2):
            qk4[2 * hh] = q_all[b, :, h0 + hh, :].T
            qk4[2 * hh + 1] = k_all[b, :, h0 + hh, :].T
        qk4 = qk4.astype(ml_dtypes.bfloat16)
        vnat = np.empty((128, 2 * NT), np.float32)
        for hh in range(2):
            # vnat[p, NT*hh + t] = v[b, 128*t + p, h0+hh]
            vnat[:, NT * hh:NT * (hh + 1)] = v[b, :, h0 + hh].reshape(NT, 128).T
        # vpat[p, 32*iv + c] = pat32[p, c] * vnat[p, iv]
        vpat = (vnat[:, :, None] * pat32[:, None, :]).reshape(
            128, 2 * NT * 32).astype(ml_dtypes.bfloat16)
        in_maps.append(dict(qk4=qk4, vpat=vpat, msk=msk))
    return in_maps


def postprocess(results):
    """Host-side gather: strip-sum, W=3 same-pool, assemble (B, L, H)."""
    S = np.zeros((H, B, L), np.float32)
    for core in range(NCORES):
        b = core // 4
        h0 = 2 * (core % 4)
        sraw = np.asarray(results[core]["sout"], np.float32)  # (2, 32, L)
        for hh in range(2):
            S[h0 + hh, b, :] = sraw[hh].sum(axis=0)
    Sp = np.pad(S, ((0, 0), (0, 0), (1, 1)))
    sums = Sp[:, :, :-2] + Sp[:, :, 1:-1] + Sp[:, :, 2:]
    counts = np.full(L, float(W), np.float32)
    counts[0] = counts[-1] = W - 1
    pooled = sums / counts[None, None, :]
    return np.ascontiguousarray(pooled.transpose(1, 2, 0)).astype(np.float32)


def kernel(x, Wq, bq, Wk, bk, Wv, pe):
    global LAST_EXEC_NS, LAST_RES
    from concourse.bass_utils import run_bass_kernel_spmd

    nc = _get_compiled()
    in_maps = make_in_maps(x, Wq, bq, Wk, bk, Wv, pe)
    res = run_bass_kernel_spmd(nc, in_maps, list(range(NCORES)), trace=TRACE)
    LAST_EXEC_NS = res.exec_time_ns
    LAST_RES = res
    return postprocess(res.results)
